# revision 45
# baseline (speedup 1.0000x reference)
"""Trainium2 Bass kernel for nn_CrossNet (topk_masking).

Algorithm (per image of 512 ROIs, 81 classes):
  L = xaug @ G_aug @ xaug^T                 (G_aug = [[s WqWk^T, s Wq bk],
                                             [s bq^T Wk^T, s bq.bk]] folded on
                                             host; biases via K=82 ones-row)
  E = exp(L); denom = sum_j E               (softmax without max-shift; logits are O(1))
  top-10 per row via threshold: e10 = 10th largest E (max8 chunk-candidate selection)
  TE = E * (E >= e10)                       (masked, unnormalized attention)
  M[j,c] = x[j,c]*(x[j,c]==max_j)           (scatter matrix: m_j at class slot label_j)
  r_un[c,i] = sum_j M[j,c] TE^T[j,i]        (matmul replaces topk gather/scatter)
  P[c,i] = prior_zd[c,label_i]/denom_i      (gathered via one-hot matmul; zero-diag
                                             prior implements the li!=lj mask)
  out^T = sigmoid(Wfc^T @ (relu(r_un)*P))   (transposed FC, one matmul; host
                                             un-transposes; sigmoid via tanh)

Structured as a 4-stage software pipeline over images so every engine works on a
different image each iteration:
  S1(b): x -> x^T -> q/k -> L -> exp      (PE+ACT ladder)
  S2(b): top-10 selection, TE, M, W       (DVE; W^T on PE)
  S3(b): TE^T transposes                  (PE + ACT copies)
  S4(b): scatter matmul, prior, FC, out   (PE + DVE + ACT)

Sharding: data-parallel over the 128-image batch, 16 images per core, weights
replicated. Inputs/outputs are full tensors; shard/gather happens on host.
"""

import sys
from contextlib import ExitStack

import numpy as np

sys.path.insert(0, "/opt/trn_rl_repo")

import ml_dtypes

import concourse.bass as bass
import concourse.tile as tile
from concourse import mybir
from concourse.bass_utils import run_bass_kernel_spmd

B, R, C, DK, RK = 128, 512, 81, 64, 10
NCORES = 8
IMG_PER_CORE = B // NCORES          # 16
ROWS_PER_CORE = IMG_PER_CORE * R    # 8192
NT = R // 128                       # 4 row-tiles per image
F32 = mybir.dt.float32
F32R = mybir.dt.float32r
BF16 = mybir.dt.bfloat16
AF = mybir.ActivationFunctionType
OP = mybir.AluOpType


def _build_bass():
    nc = bass.Bass()

    x_d = nc.dram_tensor("x", [ROWS_PER_CORE, C], F32, kind="ExternalInput")
    g_d = nc.dram_tensor("g_aug", [C + 1, C + 1], F32R, kind="ExternalInput")
    pr_d = nc.dram_tensor("prior_zdT", [C, C], BF16, kind="ExternalInput")
    wfc_d = nc.dram_tensor("wfc_pad", [C + 1, C], BF16, kind="ExternalInput")
    id_d = nc.dram_tensor("ident", [128, 128], F32, kind="ExternalInput")
    idb_d = nc.dram_tensor("ident_bf", [128, 128], BF16, kind="ExternalInput")
    ones_d = nc.dram_tensor("ones_r", [1, R], F32R, kind="ExternalInput")
    onesb_d = nc.dram_tensor("ones_b", [1, R], BF16, kind="ExternalInput")
    out_d = nc.dram_tensor("out", [IMG_PER_CORE, C, R], F32,
                           kind="ExternalOutput")

    # per-image DRAM views: [p, ic, c] with partition = row-within-chunk
    x_v = x_d.rearrange("(b ic p) c -> b p ic c", b=IMG_PER_CORE, ic=NT, p=128)

    with TileKernel(nc) as tk:
        tk.run(x_v, g_d, pr_d, wfc_d, id_d, idb_d, ones_d, onesb_d, out_d)
    # Walrus allows at most one semaphore wait per TPB instruction; these
    # bacc passes split excess matmul waits onto ldweights/event-semaphores.
    import bass_rust
    bass_rust.move_matmul_waits_to_ldweights(nc.m)
    bass_rust.generate_event_semaphores(nc)
    return nc


class TileKernel:
    def __init__(self, nc):
        self.nc = nc
        self.ctx = ExitStack()

    def __enter__(self):
        self.tc = self.ctx.enter_context(tile.TileContext(self.nc))
        return self

    def __exit__(self, *exc):
        return self.ctx.__exit__(*exc)

    def run(self, x_v, g_d, pr_d, wfc_d, id_d, idb_d, ones_d, onesb_d,
            out_d):
        nc, tc, ctx = self.nc, self.tc, self.ctx

        singles = ctx.enter_context(tc.tile_pool(name="singles", bufs=1))
        p = {}
        p["x"] = ctx.enter_context(tc.tile_pool(name="sb_x", bufs=4))
        p["qk"] = ctx.enter_context(tc.tile_pool(name="sb_qk", bufs=2))
        p["e"] = ctx.enter_context(tc.tile_pool(name="sb_e", bufs=10))
        p["te"] = ctx.enter_context(tc.tile_pool(name="sb_te", bufs=10))
        p["tet"] = ctx.enter_context(tc.tile_pool(name="sb_tet", bufs=4))
        p["dte"] = ctx.enter_context(
            tc.tile_pool(name="dr_te", bufs=4, space="DRAM"))
        p["sel"] = ctx.enter_context(tc.tile_pool(name="sb_sel", bufs=4))
        p["small"] = ctx.enter_context(tc.tile_pool(name="sb_small", bufs=4))
        p["mw"] = ctx.enter_context(tc.tile_pool(name="sb_mw", bufs=12))
        p["fc"] = ctx.enter_context(tc.tile_pool(name="sb_fc", bufs=3))
        p["out"] = ctx.enter_context(tc.tile_pool(name="sb_out", bufs=3))
        # PSUM pools (8 banks: trans 2 + qk 2 + L 2 + r 1 + p 1)
        p["trans"] = ctx.enter_context(
            tc.tile_pool(name="ps_trans", bufs=2, space="PSUM"))
        p["psqk"] = ctx.enter_context(
            tc.tile_pool(name="ps_qk", bufs=1, space="PSUM"))
        p["psl"] = ctx.enter_context(
            tc.tile_pool(name="ps_l", bufs=2, space="PSUM"))
        p["psr"] = ctx.enter_context(
            tc.tile_pool(name="ps_r", bufs=1, space="PSUM"))
        p["psp"] = ctx.enter_context(
            tc.tile_pool(name="ps_p", bufs=1, space="PSUM"))
        self.p = p

        # constants
        self.g_sb = singles.tile([C + 1, C + 1], F32R, name="g_sb")
        nc.sync.dma_start(out=self.g_sb, in_=g_d[:])
        self.pr_sb = singles.tile([C, C], BF16, name="pr_sb")
        nc.sync.dma_start(out=self.pr_sb, in_=pr_d[:])
        self.wfc_sb = singles.tile([C + 1, C], BF16, name="wfc_sb")
        nc.sync.dma_start(out=self.wfc_sb, in_=wfc_d[:])
        self.ident = singles.tile([128, 128], F32, name="ident_sb")
        nc.sync.dma_start(out=self.ident, in_=id_d[:])
        self.ident_bf = singles.tile([128, 128], BF16, name="identb_sb")
        nc.sync.dma_start(out=self.ident_bf, in_=idb_d[:])

        # static padded tiles with a DMA'd ones-row at partition C
        self.xt_static = []
        for i in range(2):
            t = singles.tile([C + 1, R], F32R, name=f"xt_st{i}")
            nc.sync.dma_start(out=t[C:C + 1, :], in_=ones_d[:])
            self.xt_static.append(t)
        self.fc_static = []
        for i in range(2):
            t = singles.tile([C + 1, R], BF16, name=f"fc_st{i}")
            nc.sync.dma_start(out=t[C:C + 1, :], in_=onesb_d[:])
            self.fc_static.append(t)

        state = [dict() for _ in range(IMG_PER_CORE)]
        self.load_x(0, x_v, state[0])
        for k in range(IMG_PER_CORE + 4):
            if k + 1 < IMG_PER_CORE:
                self.load_x(k + 1, x_v, state[k + 1])
            if k < IMG_PER_CORE:
                self.s1_logits(k, state[k])
            if 0 <= k - 1 < IMG_PER_CORE:
                self.s2_select(k - 1, state[k - 1])
            if 0 <= k - 2 < IMG_PER_CORE:
                self.s3_transpose(k - 2, state[k - 2])
            if 0 <= k - 4 < IMG_PER_CORE:
                self.s4_scatter_fc(k - 4, state[k - 4], out_d)
                state[k - 4] = None

    def load_x(self, b, x_v, st):
        x_t = self.p["x"].tile([128, NT, C], F32, name=f"x_{b}", tag="x")
        self.nc.sync.dma_start(out=x_t, in_=x_v[b])
        st["x"] = x_t

    def s1_logits(self, b, st):
        nc, p = self.nc, self.p
        x_t = st["x"]

        # x^T via PE transposes -> PSUM [81, 512] -> static padded SBUF tile
        xt_ps = p["trans"].tile([C, R], F32, name=f"xtps_{b}", tag="trans")
        for ic in range(NT):
            nc.tensor.transpose(
                out=xt_ps[:, ic * 128:(ic + 1) * 128], in_=x_t[:, ic, :],
                identity=self.ident,
            )
        xt_sb = self.xt_static[b % 2]
        nc.scalar.activation(out=xt_sb[0:C, :], in_=xt_ps, func=AF.Copy)

        # u^T = G_aug^T @ xaug^T  [82, 512] (K=82 folds both biases)
        u_ps = p["psqk"].tile([C + 1, R], F32, name=f"ups_{b}", tag="qk")
        nc.tensor.matmul(out=u_ps, lhsT=self.g_sb, rhs=xt_sb)
        u_sb = p["qk"].tile([C + 1, R], F32R, name=f"u_{b}", tag="qk")
        nc.scalar.activation(out=u_sb, in_=u_ps, func=AF.Copy)

        # logits + exp (+ per-tile denom)
        denom4 = p["small"].tile([128, NT], F32, name=f"den_{b}", tag="den")
        m4 = p["small"].tile([128, NT], F32, name=f"m4_{b}", tag="m4")
        nc.vector.tensor_reduce(
            out=m4, in_=x_t, axis=mybir.AxisListType.X, op=OP.max,
        )
        e_tiles = []
        for ic in range(NT):
            l_ps = p["psl"].tile([128, R], F32, name=f"l_{b}_{ic}", tag="l")
            nc.tensor.matmul(
                out=l_ps,
                lhsT=u_sb[:, ic * 128:(ic + 1) * 128],
                rhs=xt_sb,
            )
            e_t = p["e"].tile([128, R], BF16, name=f"e_{b}_{ic}", tag="e")
            nc.scalar.activation(
                out=e_t, in_=l_ps, func=AF.Exp,
                accum_out=denom4[:, ic:ic + 1],
            )
            e_tiles.append(e_t)
        st["e"] = e_tiles
        st["denom"] = denom4
        st["m4"] = m4

    def s2_select(self, b, st):
        nc, p = self.nc, self.p
        x_t, e_tiles, denom4, m4 = st["x"], st["e"], st["denom"], st["m4"]

        recip4 = p["small"].tile([128, NT], F32, name=f"rec_{b}", tag="rec")
        nc.vector.reciprocal(out=recip4, in_=denom4)

        te_t = p["te"].tile([128, NT, R], BF16, name=f"te_{b}", tag="te")
        for ic in range(NT):
            e_t = e_tiles[ic]
            # candidates: top8 of each 128-chunk (exact unless >8 of the true
            # top-10 share one chunk: ~1e-4 per row)
            cand = p["sel"].tile([128, 32], BF16, name=f"cand_{b}_{ic}", tag="cand")
            for kc in range(4):
                nc.vector.max(
                    out=cand[:, kc * 8:(kc + 1) * 8],
                    in_=e_t[:, kc * 128:(kc + 1) * 128],
                )
            top8 = p["sel"].tile([128, 8], BF16, name=f"top8_{b}_{ic}", tag="top8")
            nc.vector.max(out=top8, in_=cand)
            candz = p["sel"].tile([128, 32], BF16, name=f"candz_{b}_{ic}",
                                  tag="candz")
            nc.vector.match_replace(
                out=candz, in_to_replace=top8, in_values=cand, imm_value=0.0,
            )
            next8 = p["sel"].tile([128, 8], BF16, name=f"next8_{b}_{ic}",
                                  tag="next8")
            nc.vector.max(out=next8, in_=candz)

            # TE = (E >= e10) * E   (single DVE pass)
            nc.vector.scalar_tensor_tensor(
                out=te_t[:, ic, :], in0=e_t, scalar=next8[:, 1:2], in1=e_t,
                op0=OP.is_ge, op1=OP.mult,
            )
        st["e"] = None
        # stage TE in DRAM for the xbar transpose
        te_d = p["dte"].tile([R, R], BF16, name=f"ted_{b}", tag="ted")
        nc.sync.dma_start(
            out=te_d.rearrange("(ic p) j -> p ic j", p=128), in_=te_t)
        st["te_d"] = te_d

        # eqm = (x == m) once; M = eqm * x; W = eqm * recip  (batched [128,4,81])
        eqm = p["mw"].tile([128, NT, C], F32, name=f"eqm_{b}", tag="eqm")
        nc.vector.tensor_tensor(
            out=eqm, in0=x_t, in1=m4.to_broadcast([128, NT, C]),
            op=OP.is_equal,
        )
        m_all = p["mw"].tile([128, NT, C], BF16, name=f"m_{b}", tag="mm")
        nc.gpsimd.tensor_tensor(out=m_all, in0=eqm, in1=x_t, op=OP.mult)
        st["m"] = [m_all[:, jc, :] for jc in range(NT)]

        w4 = p["mw"].tile([128, NT, 128], BF16, name=f"w4_{b}", tag="wsrc")
        nc.gpsimd.memset(w4[:, :, C:128], 0.0)
        nc.gpsimd.tensor_tensor(
            out=w4[:, :, 0:C], in0=eqm,
            in1=recip4.to_broadcast([128, NT, C]), op=OP.mult,
        )
        wt_ps = p["trans"].tile([128, R], BF16, name=f"wtps_{b}", tag="trans")
        for ic in range(NT):
            nc.tensor.transpose(
                out=wt_ps[:, ic * 128:(ic + 1) * 128], in_=w4[:, ic, :],
                identity=self.ident_bf,
            )
        wt_sb = p["mw"].tile([128, R], BF16, name=f"wt_{b}", tag="wt")
        nc.scalar.activation(out=wt_sb, in_=wt_ps, func=AF.Copy)
        st["wt"] = wt_sb

    def s3_transpose(self, b, st):
        nc, p = self.nc, self.p
        # tw[:, jc, i] = TE[i, jc*128+p]; two xbar calls so the data lands
        # on two parallel DMA engines
        tw = p["tet"].tile([128, NT, R], BF16, name=f"tw_{b}", tag="tet")
        for h in range(2):
            nc.sync.dma_start_transpose(
                out=tw[:, 2 * h:2 * h + 2, :],
                in_=st["te_d"][:, 256 * h:256 * (h + 1)],
            )
        st["tw"] = tw
        st["te_d"] = None

    def s4_scatter_fc(self, b, st, out_d):
        nc, p = self.nc, self.p

        # r_un^T [81, 512] += M[jc].T @ TE^T[jc]  (accumulate over jc)
        r_ps = p["psr"].tile([C, R], F32, name=f"rps_{b}", tag="r")
        for jc in range(NT):
            nc.tensor.matmul(
                out=r_ps,
                lhsT=st["m"][jc],
                rhs=st["tw"][:, jc, :],
                start=(jc == 0), stop=(jc == NT - 1),
            )

        # P^T [81, 512] = prior_zdT.T @ W^T
        p_ps = p["psp"].tile([C, R], F32, name=f"pps_{b}", tag="p")
        nc.tensor.matmul(out=p_ps, lhsT=self.pr_sb, rhs=st["wt"][0:C, :])
        pt_sb = p["fc"].tile([C, R], BF16, name=f"pt_{b}", tag="pt")
        nc.scalar.activation(out=pt_sb, in_=p_ps, func=AF.Copy)
        r_relu = p["fc"].tile([C, R], BF16, name=f"rrelu_{b}", tag="rrelu")
        nc.scalar.activation(out=r_relu, in_=r_ps, func=AF.Relu)

        # fc_in = relu(r) * P into the static padded tile (ones-row at C)
        fc_in = self.fc_static[b % 2]
        nc.gpsimd.tensor_tensor(
            out=fc_in[0:C, :], in0=r_relu, in1=pt_sb, op=OP.mult,
        )

        # fc logits transposed [81, 512] in one matmul (K=82 folds bias);
        # the host un-transposes the per-image [C, R] output.
        fc_ps = p["trans"].tile([C, R], F32, name=f"fcps_{b}", tag="trans")
        nc.tensor.matmul(out=fc_ps, lhsT=self.wfc_sb, rhs=fc_in)

        # sigmoid via tanh: out = 0.5 + 0.5*tanh(0.5*logits)
        sig = p["out"].tile([C, R], F32, name=f"sig_{b}", tag="sig")
        nc.scalar.activation(out=sig, in_=fc_ps, func=AF.Tanh, scale=0.5)
        o_t = p["out"].tile([C, R], F32, name=f"o_{b}", tag="o")
        nc.vector.tensor_scalar(o_t, sig, 1.0, 0.5, op0=OP.add, op1=OP.mult)
        nc.sync.dma_start(out=out_d[b], in_=o_t)


def _install_ntff_hook():
    """Provide antenv.axon_hooks if the image lacks it (profiling only)."""
    import types
    try:
        from antenv.axon_hooks import get_axon_ntff_profile_hook  # noqa: F401
        return
    except ImportError:
        pass
    try:
        from trn_agent_boot.trn_boot import _ntff_profile_via_ctypes
        hook = _ntff_profile_via_ctypes("/opt/axon/libaxon_pjrt.so")
    except Exception:
        hook = None
    mod = types.ModuleType("antenv.axon_hooks")
    mod.get_axon_ntff_profile_hook = lambda: hook
    mod.set_axon_ntff_profile_hook = lambda h: None
    sys.modules["antenv.axon_hooks"] = mod


_NC_CACHE = None


def _get_nc():
    global _NC_CACHE
    if _NC_CACHE is None:
        _NC_CACHE = _build_bass()
    return _NC_CACHE


def kernel(x, Wq, bq, Wk, bk, Wfc, bfc, prior_rel, _trace=False):
    x = np.ascontiguousarray(np.asarray(x, np.float32))
    Wq = np.asarray(Wq, np.float32); bq = np.asarray(bq, np.float32)
    Wk = np.asarray(Wk, np.float32); bk = np.asarray(bk, np.float32)
    Wfc = np.asarray(Wfc, np.float32); bfc = np.asarray(bfc, np.float32)
    prior = np.asarray(prior_rel, np.float32)

    s = np.float32(1.0 / np.sqrt(np.float32(DK)))
    g_aug = np.zeros((C + 1, C + 1), np.float32)
    g_aug[:C, :C] = s * (Wq @ Wk.T)
    g_aug[:C, C] = s * (Wq @ bk)
    g_aug[C, :C] = s * (Wk @ bq)
    g_aug[C, C] = s * float(bq @ bk)
    prior_zd = prior.copy()
    np.fill_diagonal(prior_zd, 0.0)
    prior_zdT = np.ascontiguousarray(prior_zd.T).astype(ml_dtypes.bfloat16)
    wfc_pad = np.vstack([Wfc, bfc[None, :]]).astype(ml_dtypes.bfloat16)

    if _trace:
        sys.path.insert(0, "/root/.axon_site")
        _install_ntff_hook()
    nc = _get_nc()
    in_maps = []
    for c in range(NCORES):
        shard = x[c * ROWS_PER_CORE:(c + 1) * ROWS_PER_CORE]
        in_maps.append({
            "x": shard,
            "g_aug": g_aug,
            "prior_zdT": prior_zdT,
            "wfc_pad": wfc_pad,
            "ident": np.eye(128, dtype=np.float32),
            "ident_bf": np.eye(128, dtype=ml_dtypes.bfloat16),
            "ones_r": np.ones((1, R), np.float32),
            "ones_b": np.ones((1, R), ml_dtypes.bfloat16),
        })
    res = run_bass_kernel_spmd(nc, in_maps, list(range(NCORES)), trace=_trace)
    out = np.concatenate(
        [np.asarray(r["out"]).transpose(0, 2, 1).reshape(-1, C)
         for r in res.results], axis=0)
    if _trace:
        return out.astype(np.float32), res
    return out.astype(np.float32)


if __name__ == "__main__":
    rng = np.random.default_rng(0)
    inputs = {
        "x": rng.standard_normal((B * R, C), dtype=np.float32),
        "Wq": rng.standard_normal((C, DK), dtype=np.float32) / 9.0,
        "bq": np.zeros(DK, np.float32),
        "Wk": rng.standard_normal((C, DK), dtype=np.float32) / 9.0,
        "bk": np.zeros(DK, np.float32),
        "Wfc": rng.standard_normal((C, C), dtype=np.float32) / 9.0,
        "bfc": np.zeros(C, np.float32),
        "prior_rel": rng.random((C, C), dtype=np.float32),
    }
    out = kernel(**inputs)
    print("out", out.shape, out.dtype, float(out.mean()))

